# revision 1
# baseline (speedup 1.0000x reference)
"""BitLinear (quantized-activation, binarized-sprinkled-weight linear) Trainium2 kernel.

Data-parallel over the token dim N across 8 NeuronCores. Each core:
  * quantize-blends its x shard with one fused custom DVE op  -> xb bf16
  * sign/sprinkle/blends the full weight with one fused custom DVE op -> W2 bf16
    (final_scale, post_bin_scale and the activation blend scale folded into
     per-partition scalars)
  * xbar-DMA-transposes both to put the contraction dim on partitions
  * 512 bf16 matmuls (K=128, N=512) + a K=1 ones x bias matmul per PSUM group
  * ScalarE copies PSUM->SBUF, DMA out.

Math: reference out = xq @ w_final^T * fs + fb with
  xq      = 0.5*x + 0.5*s*clip(round(x/(s+eps)), +-127)        (s = running_max/127)
  w_final = m ? h : 0.5*(w + h),  h = sign(w)*pbs
Here  xb = x*inv_se + clip(round(x*inv_se), +-127)  with inv_se = 1/(s+eps), so
  xq ~= sigma*xb with sigma = 0.5*(s+eps)   (error <= 0.5*eps*127 ~ 6e-5 absolute)
and the device computes  out = xb @ W2^T + fb  with W2 = sigma*fs*w_final:
  W2 = m ? sign(w)*C0 : w*C1 + sign(w)*C0*0.5,
  C0[o] = sigma*fs[o]*pbs[o],  C1[o] = 0.5*sigma*fs[o].
"""

import numpy as np

N_CORES = 8
N_TOK, D_IN, D_OUT = 8192, 2048, 2048
N_SHARD = N_TOK // N_CORES          # 1024 rows of x per core
P = 128
NJ = N_SHARD // P                   # 8 n-blocks per core
NB_I = D_IN // P                    # 16 i-blocks (contraction)
NB_O = D_OUT // P                   # 16 o-blocks
OT = 512                            # o-tile (one PSUM bank)
NT = D_OUT // OT                    # 4 o-tiles
OB_PER_T = OT // P                  # 4 o-blocks per o-tile

QMAX = 127.0
EPS = 1e-6
MAGIC = 12582912.0                  # 1.5 * 2**23: fp32 RNE round-to-int trick

_CACHE = {}


def _register_ops():
    """Define the two fused DVE ops (idempotent)."""
    from concourse import dve_ops
    from concourse.dve_spec import (
        Spec, Src0, Src1, C0, C1, C2, Zero, select, minn, maxx, lower, _has_src1,
    )
    from concourse.dve_uop import DveOpSpec

    def register(name, spec):
        for op in dve_ops.OPS:
            if op.name == name:
                return op
        ver = "v3"
        tmp = DveOpSpec(name=name, opcode=0, uops=lower(spec, ver=ver),
                        rd1_en=_has_src1(spec))
        op = dve_ops.DveOp(name, spec, subdim=False,
                           uops_sha={ver: tmp.sha(ver)})
        dve_ops.OPS.append(op)
        dve_ops._SUB_OPCODE_FOR_NAME[name] = (
            max(dve_ops._SUB_OPCODE_FOR_NAME.values()) + 1)
        dve_ops.CUSTOM_DVE_SPECS[name] = spec
        return op

    # out = t + clip(round(t), +-imm2), t = x*s0   (s1 = MAGIC)
    _t = Src0 * C0
    _r = (_t + C1) - C1
    _rc = minn(maxx(_r, Zero - C2), C2)
    xprep = register("XPREP_BITLIN", Spec(
        body=_t + _rc,
        reference=lambda in0, in1, s0, s1, imm2: (
            (lambda t: t + np.clip(np.round(t), -imm2, imm2))(
                in0.astype(np.float32) * s0)),
    ))

    # h = select(w>=0, s0, -s0); out = select(m>0, h, w*s1 + h*imm2)
    _h = select(Src0 >= Zero, C0, Zero - C0)
    wprep = register("WPREP_BITLIN", Spec(
        body=select(Src1 > Zero, _h, Src0 * C1 + _h * C2),
        reference=lambda in0, in1, s0, s1, imm2: (
            (lambda h: np.where(in1 > 0, h,
                                in0.astype(np.float32) * s1 + h * imm2))(
                np.where(in0 >= 0, s0, -s0))),
    ))
    return xprep, wprep


def _build(inv_se):
    """Build + compile the per-core Bass module. inv_se is baked in."""
    key = ("nc", float(inv_se))
    if key in _CACHE:
        return _CACHE[key]

    import concourse.mybir as mybir
    import concourse.tile as tile
    from concourse import bacc

    xprep, wprep = _register_ops()

    nc = bacc.Bacc(None, target_bir_lowering=False)
    bf16 = mybir.dt.bfloat16
    f32 = mybir.dt.float32

    x_in = nc.dram_tensor("x", [N_SHARD, D_IN], f32, kind="ExternalInput")
    w_in = nc.dram_tensor("w", [D_OUT, D_IN], f32, kind="ExternalInput")
    m_in = nc.dram_tensor("m", [D_OUT, D_IN], mybir.dt.uint8, kind="ExternalInput")
    c0_in = nc.dram_tensor("c0", [P, NB_O], f32, kind="ExternalInput")
    c1_in = nc.dram_tensor("c1", [P, NB_O], f32, kind="ExternalInput")
    fb_in = nc.dram_tensor("fb", [1, D_OUT], f32, kind="ExternalInput")
    out_o = nc.dram_tensor("out", [N_SHARD, D_OUT], f32, kind="ExternalOutput")

    with tile.TileContext(nc) as tc:
        with (
            tc.tile_pool(name="persist", bufs=1) as persist,
            tc.tile_pool(name="wstage", bufs=3) as wstage,
            tc.tile_pool(name="xstage", bufs=3) as xstage,
            tc.tile_pool(name="ostage", bufs=4) as ostage,
            tc.tile_pool(name="psum", bufs=4, space="PSUM") as psum,
        ):
            # persistent operand tiles
            wT = persist.tile([P, NB_O, NB_I, P], bf16, tag="wT")     # [i_in, ob, ib, o_in]
            xqT = persist.tile([P, NJ, NB_I, P], bf16, tag="xqT")     # [i_in, j, ib, n_in]
            c0_sb = persist.tile([P, NB_O], f32, tag="c0")
            c1_sb = persist.tile([P, NB_O], f32, tag="c1")
            fb_sb = persist.tile([1, D_OUT], bf16, tag="fb")
            ones = persist.tile([1, P], bf16, tag="ones")

            nc.sync.dma_start(c0_sb[:], c0_in[:])
            nc.sync.dma_start(c1_sb[:], c1_in[:])
            nc.gpsimd.dma_start(fb_sb[:], fb_in[:])       # f32 -> bf16 cast
            nc.gpsimd.memset(ones[:], 1.0)

            def w_block(ob):
                wt = wstage.tile([P, D_IN], f32, tag="w_f32")
                mt = wstage.tile([P, D_IN], bf16, tag="m_bf16")
                nc.sync.dma_start(wt[:], w_in[ob * P:(ob + 1) * P, :])
                nc.gpsimd.dma_start(mt[:], m_in[ob * P:(ob + 1) * P, :])  # u8->bf16
                w2 = wstage.tile([P, D_IN], bf16, tag="w2")
                nc.vector._custom_dve(
                    wprep, out=w2[:], in0=wt[:], in1=mt[:],
                    s0=c0_sb[:, ob:ob + 1], s1=c1_sb[:, ob:ob + 1], imm2=0.5)
                nc.sync.dma_start_transpose(wT[:, ob], w2[:])

            def x_block(j):
                xt = xstage.tile([P, D_IN], f32, tag="x_f32")
                nc.sync.dma_start(xt[:], x_in[j * P:(j + 1) * P, :])
                xb = xstage.tile([P, D_IN], bf16, tag="xb")
                nc.vector._custom_dve(
                    xprep, out=xb[:], in0=xt[:],
                    s0=float(inv_se), s1=MAGIC, imm2=QMAX)
                nc.sync.dma_start_transpose(xqT[:, j], xb[:])

            # weight blocks for the first o-tile, then x, then the rest
            for ob in range(OB_PER_T):
                w_block(ob)
            for j in range(NJ):
                x_block(j)
            for ob in range(OB_PER_T, NB_O):
                w_block(ob)

            for t in range(NT):
                for j in range(NJ):
                    ps = psum.tile([P, OT], f32, tag="ps")
                    nc.tensor.matmul(ps[:], ones[:],
                                     fb_sb[:, t * OT:(t + 1) * OT],
                                     start=True, stop=False)
                    for b in range(NB_I):
                        nc.tensor.matmul(
                            ps[:],
                            xqT[:, j, b, :],
                            wT[:, t * OB_PER_T:(t + 1) * OB_PER_T, b, :],
                            start=False, stop=(b == NB_I - 1))
                    osb = ostage.tile([P, OT], f32, tag="osb")
                    nc.scalar.copy(osb[:], ps[:])
                    nc.sync.dma_start(
                        out_o[j * P:(j + 1) * P, t * OT:(t + 1) * OT], osb[:])

    nc.compile()
    _CACHE[key] = nc
    return nc


def _in_maps(x, weight, mask_u8, c0, c1, fb):
    maps = []
    for c in range(N_CORES):
        maps.append({
            "x": np.ascontiguousarray(x[c * N_SHARD:(c + 1) * N_SHARD]),
            "w": weight,
            "m": mask_u8,
            "c0": c0,
            "c1": c1,
            "fb": fb,
        })
    return maps


def _host_consts(post_bin_scale, final_scale, final_bias, running_max):
    s = np.float32(running_max) / np.float32(QMAX)
    inv_se = np.float32(1.0) / (s + np.float32(EPS))
    sigma = np.float64(0.5) * (np.float64(s) + np.float64(EPS))
    c0_all = (sigma * final_scale.astype(np.float64)
              * post_bin_scale.reshape(-1).astype(np.float64)).astype(np.float32)
    c1_all = (np.float64(0.5) * sigma
              * final_scale.astype(np.float64)).astype(np.float32)
    # [o] -> [p, ob] with o = ob*128 + p
    c0 = np.ascontiguousarray(c0_all.reshape(NB_O, P).T)
    c1 = np.ascontiguousarray(c1_all.reshape(NB_O, P).T)
    fb = np.ascontiguousarray(final_bias.astype(np.float32).reshape(1, D_OUT))
    return inv_se, c0, c1, fb


def kernel(x, weight, post_bin_scale, final_scale, final_bias, running_max,
           sprinkle_mask):
    from concourse.bass_utils import run_bass_kernel_spmd

    x = np.asarray(x, dtype=np.float32)
    weight = np.ascontiguousarray(np.asarray(weight, dtype=np.float32))
    mask_u8 = np.ascontiguousarray(np.asarray(sprinkle_mask)).view(np.uint8)
    inv_se, c0, c1, fb = _host_consts(
        np.asarray(post_bin_scale, dtype=np.float32),
        np.asarray(final_scale, dtype=np.float32),
        np.asarray(final_bias, dtype=np.float32),
        float(np.asarray(running_max)))

    nc = _build(inv_se)
    maps = _in_maps(x, weight, mask_u8, c0, c1, fb)
    res = run_bass_kernel_spmd(nc, maps, core_ids=list(range(N_CORES)))
    out = np.concatenate([res.results[c]["out"] for c in range(N_CORES)], axis=0)
    return out


# revision 3
# speedup vs baseline: 30646.0083x; 30646.0083x over previous
"""BitLinear (quantized-activation, binarized-sprinkled-weight linear) Trainium2 kernel.

Data-parallel over the token dim N across 8 NeuronCores. Each core:
  * quantize-blends its x shard with one fused custom DVE op  -> xb bf16
  * sign/sprinkle/blends the full weight with one fused custom DVE op -> W2 bf16
    (final_scale, post_bin_scale and the activation blend scale folded into
     per-partition scalars; weight DMA-cast to bf16 on load; mask read as u8)
  * xbar-DMA-transposes both to put the contraction dim on partitions
  * 512 bf16 matmuls (K=128, N=512) accumulating in PSUM
  * DVE adds the broadcast final_bias while copying PSUM->SBUF, DMA out.

Math: reference out = xq @ w_final^T * fs + fb with
  xq      = 0.5*x + 0.5*s*clip(round(x/(s+eps)), +-127)        (s = running_max/127)
  w_final = m ? h : 0.5*(w + h),  h = sign(w)*pbs
Here  xb = x*inv_se + clip(round(x*inv_se), +-127)  with inv_se = 1/(s+eps), so
  xq ~= sigma*xb with sigma = 0.5*(s+eps)   (error <= 0.5*eps*127 ~ 6e-5 absolute)
and the device computes  out = xb @ W2^T + fb  with W2 = sigma*fs*w_final:
  W2 = m ? sign(w)*C0 : w*C1 + sign(w)*C0*0.5,
  C0[o] = sigma*fs[o]*pbs[o],  C1[o] = 0.5*sigma*fs[o].
"""

import numpy as np

N_CORES = 8
N_TOK, D_IN, D_OUT = 8192, 2048, 2048
N_SHARD = N_TOK // N_CORES          # 1024 rows of x per core
P = 128
NJ = N_SHARD // P                   # 8 n-blocks per core
NB_I = D_IN // P                    # 16 i-blocks (contraction)
NB_O = D_OUT // P                   # 16 o-blocks
OT = 512                            # o-tile (one PSUM bank)
NT = D_OUT // OT                    # 4 o-tiles
OB_PER_T = OT // P                  # 4 o-blocks per o-tile

QMAX = 127.0
EPS = 1e-6
MAGIC = 12582912.0                  # 1.5 * 2**23: fp32 RNE round-to-int trick

_CACHE = {}


def _register_ops():
    """Define the two fused DVE ops (idempotent)."""
    from concourse import dve_ops
    from concourse.dve_spec import (
        Spec, Src0, Src1, C0, C1, C2, Zero, select, minn, maxx, lower, _has_src1,
    )
    from concourse.dve_uop import DveOpSpec

    def register(name, spec):
        for op in dve_ops.OPS:
            if op.name == name:
                return op
        ver = "v3"
        tmp = DveOpSpec(name=name, opcode=0, uops=lower(spec, ver=ver),
                        rd1_en=_has_src1(spec))
        op = dve_ops.DveOp(name, spec, subdim=False,
                           uops_sha={ver: tmp.sha(ver)})
        dve_ops.OPS.append(op)
        dve_ops._SUB_OPCODE_FOR_NAME[name] = (
            max(dve_ops._SUB_OPCODE_FOR_NAME.values()) + 1)
        dve_ops.CUSTOM_DVE_SPECS[name] = spec
        return op

    # out = t + clip(round(t), +-imm2), t = x*s0   (s1 = MAGIC)
    _t = Src0 * C0
    _r = (_t + C1) - C1
    _rc = minn(maxx(_r, Zero - C2), C2)
    xprep = register("XPREP_BITLIN", Spec(
        body=_t + _rc,
        reference=lambda in0, in1, s0, s1, imm2: (
            (lambda t: t + np.clip(np.round(t), -imm2, imm2))(
                in0.astype(np.float32) * s0)),
    ))

    # h = select(w>=0, s0, -s0); out = select(m>0, h, w*s1 + h*imm2)
    _h = select(Src0 >= Zero, C0, Zero - C0)
    wprep = register("WPREP_BITLIN", Spec(
        body=select(Src1 > Zero, _h, Src0 * C1 + _h * C2),
        reference=lambda in0, in1, s0, s1, imm2: (
            (lambda h: np.where(in1 > 0, h,
                                in0.astype(np.float32) * s1 + h * imm2))(
                np.where(in0 >= 0, s0, -s0))),
    ))
    return xprep, wprep


def _build(inv_se):
    """Build + compile the per-core Bass module. inv_se is baked in."""
    key = ("nc", float(inv_se))
    if key in _CACHE:
        return _CACHE[key]

    import concourse.mybir as mybir
    import concourse.tile as tile
    from concourse import bacc

    xprep, wprep = _register_ops()

    nc = bacc.Bacc(None, target_bir_lowering=False)
    bf16 = mybir.dt.bfloat16
    f32 = mybir.dt.float32

    x_in = nc.dram_tensor("x", [N_SHARD, D_IN], f32, kind="ExternalInput")
    w_in = nc.dram_tensor("w", [D_OUT, D_IN], f32, kind="ExternalInput")
    m_in = nc.dram_tensor("m", [D_OUT, D_IN], mybir.dt.uint8, kind="ExternalInput")
    c0_in = nc.dram_tensor("c0", [P, NB_O], f32, kind="ExternalInput")
    c1_in = nc.dram_tensor("c1", [P, NB_O], f32, kind="ExternalInput")
    fb_in = nc.dram_tensor("fb", [P, D_OUT], f32, kind="ExternalInput")
    out_o = nc.dram_tensor("out", [N_SHARD, D_OUT], f32, kind="ExternalOutput")

    from concourse.masks import make_identity

    with tile.TileContext(nc) as tc:
        with (
            tc.tile_pool(name="persist", bufs=1) as persist,
            tc.tile_pool(name="wlp", bufs=5) as wlp,
            tc.tile_pool(name="wpp", bufs=4) as wpp,
            tc.tile_pool(name="xlp", bufs=3) as xlp,
            tc.tile_pool(name="xbp", bufs=4) as xbp,
            tc.tile_pool(name="ostage", bufs=6) as ostage,
            tc.tile_pool(name="psum", bufs=6, space="PSUM") as psum,
            tc.tile_pool(name="tpsum", bufs=2, space="PSUM") as tpsum,
        ):
            # persistent operand tiles
            wT = persist.tile([P, NB_O, NB_I, P], bf16, tag="wT")     # [i_in, ob, ib, o_in]
            xqT = persist.tile([P, NJ, NB_I, P], bf16, tag="xqT")     # [i_in, j, ib, n_in]
            c0_sb = persist.tile([P, NB_O], f32, tag="c0")
            c1_sb = persist.tile([P, NB_O], f32, tag="c1")
            fb_sb = persist.tile([P, D_OUT], f32, tag="fb")
            ident = persist.tile([P, P], bf16, tag="ident")

            nc.sync.dma_start(fb_sb[:], fb_in[:])
            nc.sync.dma_start(c0_sb[:], c0_in[:])
            nc.sync.dma_start(c1_sb[:], c1_in[:])
            make_identity(nc, ident[:])

            def w_block(ob):
                wt = wlp.tile([P, D_IN], bf16, tag="w_bf16")
                mt = wlp.tile([P, D_IN], mybir.dt.uint8, tag="m_u8")
                nc.gpsimd.dma_start(wt[:], w_in[ob * P:(ob + 1) * P, :])   # f32->bf16
                nc.scalar.dma_start(mt[:], m_in[ob * P:(ob + 1) * P, :])
                w2 = wpp.tile([P, D_IN], bf16, tag="w2")
                nc.vector._custom_dve(
                    wprep, out=w2[:], in0=wt[:], in1=mt[:],
                    s0=c0_sb[:, ob:ob + 1], s1=c1_sb[:, ob:ob + 1], imm2=0.5)
                nc.sync.dma_start_transpose(wT[:, ob], w2[:])

            def x_block(j):
                xt = xlp.tile([P, D_IN], f32, tag="x_f32")
                nc.sync.dma_start(xt[:], x_in[j * P:(j + 1) * P, :])
                xb = xbp.tile([P, D_IN], bf16, tag="xb")
                nc.vector._custom_dve(
                    xprep, out=xb[:], in0=xt[:],
                    s0=float(inv_se), s1=MAGIC, imm2=QMAX)
                # transpose on the (otherwise idle-at-this-point) PE + ScalarE
                for b in range(NB_I):
                    tp = tpsum.tile([P, P], bf16, tag="xtp")
                    nc.tensor.transpose(tp[:], xb[:, b * P:(b + 1) * P], ident[:])
                    nc.scalar.copy(xqT[:, j, b, :], tp[:])

            # weight blocks for the first o-tile, then x, then the rest
            for ob in range(OB_PER_T):
                w_block(ob)
            for j in range(NJ):
                x_block(j)
            for ob in range(OB_PER_T, NB_O):
                w_block(ob)

            for t in range(NT):
                for j in range(NJ):
                    ps = psum.tile([P, OT], f32, tag="ps")
                    for b in range(NB_I):
                        nc.tensor.matmul(
                            ps[:],
                            xqT[:, j, b, :],
                            wT[:, t * OB_PER_T:(t + 1) * OB_PER_T, b, :],
                            start=(b == 0), stop=(b == NB_I - 1))
                    osb = ostage.tile([P, OT], f32, tag="osb")
                    nc.vector.tensor_add(
                        osb[:], ps[:], fb_sb[:, t * OT:(t + 1) * OT])
                    nc.scalar.dma_start(
                        out_o[j * P:(j + 1) * P, t * OT:(t + 1) * OT], osb[:])

    nc.compile()
    _CACHE[key] = nc
    return nc


def _in_maps(x, weight, mask_u8, c0, c1, fb):
    maps = []
    for c in range(N_CORES):
        maps.append({
            "x": np.ascontiguousarray(x[c * N_SHARD:(c + 1) * N_SHARD]),
            "w": weight,
            "m": mask_u8,
            "c0": c0,
            "c1": c1,
            "fb": fb,
        })
    return maps


def _host_consts(post_bin_scale, final_scale, final_bias, running_max):
    s = np.float32(running_max) / np.float32(QMAX)
    inv_se = np.float32(1.0) / (s + np.float32(EPS))
    sigma = np.float64(0.5) * (np.float64(s) + np.float64(EPS))
    c0_all = (sigma * final_scale.astype(np.float64)
              * post_bin_scale.reshape(-1).astype(np.float64)).astype(np.float32)
    c1_all = (np.float64(0.5) * sigma
              * final_scale.astype(np.float64)).astype(np.float32)
    # [o] -> [p, ob] with o = ob*128 + p
    c0 = np.ascontiguousarray(c0_all.reshape(NB_O, P).T)
    c1 = np.ascontiguousarray(c1_all.reshape(NB_O, P).T)
    fb = np.ascontiguousarray(
        np.broadcast_to(final_bias.astype(np.float32), (P, D_OUT)))
    return inv_se, c0, c1, fb


def kernel(x, weight, post_bin_scale, final_scale, final_bias, running_max,
           sprinkle_mask):
    from concourse.bass_utils import run_bass_kernel_spmd

    x = np.asarray(x, dtype=np.float32)
    weight = np.ascontiguousarray(np.asarray(weight, dtype=np.float32))
    mask_u8 = np.ascontiguousarray(np.asarray(sprinkle_mask)).view(np.uint8)
    inv_se, c0, c1, fb = _host_consts(
        np.asarray(post_bin_scale, dtype=np.float32),
        np.asarray(final_scale, dtype=np.float32),
        np.asarray(final_bias, dtype=np.float32),
        float(np.asarray(running_max)))

    nc = _build(inv_se)
    maps = _in_maps(x, weight, mask_u8, c0, c1, fb)
    res = run_bass_kernel_spmd(nc, maps, core_ids=list(range(N_CORES)))
    out = np.concatenate([res.results[c]["out"] for c in range(N_CORES)], axis=0)
    return out


# revision 4
# speedup vs baseline: 32282.9891x; 1.0534x over previous
"""BitLinear (quantized-activation, binarized-sprinkled-weight linear) Trainium2 kernel.

Data-parallel over the token dim N across 8 NeuronCores. Each core:
  * quantize-blends its x shard with one fused custom DVE op  -> xb bf16
  * sign/sprinkle/blends the full weight with one fused custom DVE op -> W2 bf16
    (final_scale, post_bin_scale and the activation blend scale folded into
     per-partition scalars; weight DMA-cast to bf16 on load; mask read as u8)
  * xbar-DMA-transposes both to put the contraction dim on partitions
  * 512 bf16 matmuls (K=128, N=512) accumulating in PSUM
  * DVE adds the broadcast final_bias while copying PSUM->SBUF, DMA out.

Math: reference out = xq @ w_final^T * fs + fb with
  xq      = 0.5*x + 0.5*s*clip(round(x/(s+eps)), +-127)        (s = running_max/127)
  w_final = m ? h : 0.5*(w + h),  h = sign(w)*pbs
Here  xb = x*inv_se + clip(round(x*inv_se), +-127)  with inv_se = 1/(s+eps), so
  xq ~= sigma*xb with sigma = 0.5*(s+eps)   (error <= 0.5*eps*127 ~ 6e-5 absolute)
and the device computes  out = xb @ W2^T + fb  with W2 = sigma*fs*w_final:
  W2 = m ? sign(w)*C0 : w*C1 + sign(w)*C0*0.5,
  C0[o] = sigma*fs[o]*pbs[o],  C1[o] = 0.5*sigma*fs[o].
"""

import numpy as np

N_CORES = 8
N_TOK, D_IN, D_OUT = 8192, 2048, 2048
N_SHARD = N_TOK // N_CORES          # 1024 rows of x per core
P = 128
NJ = N_SHARD // P                   # 8 n-blocks per core
NB_I = D_IN // P                    # 16 i-blocks (contraction)
NB_O = D_OUT // P                   # 16 o-blocks
OT = 512                            # o-tile (one PSUM bank)
NT = D_OUT // OT                    # 4 o-tiles
OB_PER_T = OT // P                  # 4 o-blocks per o-tile

QMAX = 127.0
EPS = 1e-6
MAGIC = 12582912.0                  # 1.5 * 2**23: fp32 RNE round-to-int trick

_CACHE = {}


def _register_ops():
    """Define the two fused DVE ops (idempotent)."""
    from concourse import dve_ops
    from concourse.dve_spec import (
        Spec, Src0, Src1, C0, C1, C2, Zero, select, minn, maxx, lower, _has_src1,
    )
    from concourse.dve_uop import DveOpSpec

    def register(name, spec):
        for op in dve_ops.OPS:
            if op.name == name:
                return op
        ver = "v3"
        tmp = DveOpSpec(name=name, opcode=0, uops=lower(spec, ver=ver),
                        rd1_en=_has_src1(spec))
        op = dve_ops.DveOp(name, spec, subdim=False,
                           uops_sha={ver: tmp.sha(ver)})
        dve_ops.OPS.append(op)
        dve_ops._SUB_OPCODE_FOR_NAME[name] = (
            max(dve_ops._SUB_OPCODE_FOR_NAME.values()) + 1)
        dve_ops.CUSTOM_DVE_SPECS[name] = spec
        return op

    # out = t + clip(round(t), +-imm2), t = x*s0   (s1 = MAGIC)
    _t = Src0 * C0
    _r = (_t + C1) - C1
    _rc = minn(maxx(_r, Zero - C2), C2)
    xprep = register("XPREP_BITLIN", Spec(
        body=_t + _rc,
        reference=lambda in0, in1, s0, s1, imm2: (
            (lambda t: t + np.clip(np.round(t), -imm2, imm2))(
                in0.astype(np.float32) * s0)),
    ))

    # h = select(w>=0, s0, -s0); out = select(m>0, h, w*s1 + h*imm2)
    _h = select(Src0 >= Zero, C0, Zero - C0)
    wprep = register("WPREP_BITLIN", Spec(
        body=select(Src1 > Zero, _h, Src0 * C1 + _h * C2),
        reference=lambda in0, in1, s0, s1, imm2: (
            (lambda h: np.where(in1 > 0, h,
                                in0.astype(np.float32) * s1 + h * imm2))(
                np.where(in0 >= 0, s0, -s0))),
    ))
    return xprep, wprep


def _build(inv_se):
    """Build + compile the per-core Bass module. inv_se is baked in."""
    key = ("nc", float(inv_se))
    if key in _CACHE:
        return _CACHE[key]

    import concourse.mybir as mybir
    import concourse.tile as tile
    from concourse import bacc

    xprep, wprep = _register_ops()

    nc = bacc.Bacc(None, target_bir_lowering=False)
    bf16 = mybir.dt.bfloat16
    f32 = mybir.dt.float32

    x_in = nc.dram_tensor("x", [N_SHARD, D_IN], f32, kind="ExternalInput")
    w_in = nc.dram_tensor("w", [D_OUT, D_IN], f32, kind="ExternalInput")
    m_in = nc.dram_tensor("m", [D_OUT, D_IN], mybir.dt.uint8, kind="ExternalInput")
    c0_in = nc.dram_tensor("c0", [P, NB_O], f32, kind="ExternalInput")
    c1_in = nc.dram_tensor("c1", [P, NB_O], f32, kind="ExternalInput")
    fb_in = nc.dram_tensor("fb", [P, D_OUT], f32, kind="ExternalInput")
    out_o = nc.dram_tensor("out", [N_SHARD, D_OUT], f32, kind="ExternalOutput")

    from concourse.masks import make_identity

    with tile.TileContext(nc) as tc:
        with (
            tc.tile_pool(name="persist", bufs=1) as persist,
            tc.tile_pool(name="wlp", bufs=4) as wlp,
            tc.tile_pool(name="wpp", bufs=4) as wpp,
            tc.tile_pool(name="xlp", bufs=4) as xlp,
            tc.tile_pool(name="xbp", bufs=4) as xbp,
            tc.tile_pool(name="ostage", bufs=6) as ostage,
            tc.tile_pool(name="psum", bufs=6, space="PSUM") as psum,
            tc.tile_pool(name="tpsum", bufs=2, space="PSUM") as tpsum,
        ):
            # persistent operand tiles
            wT = persist.tile([P, NB_O, NB_I, P], bf16, tag="wT")     # [i_in, ob, ib, o_in]
            xqT = persist.tile([P, NJ, NB_I, P], bf16, tag="xqT")     # [i_in, j, ib, n_in]
            c0_sb = persist.tile([P, NB_O], f32, tag="c0")
            c1_sb = persist.tile([P, NB_O], f32, tag="c1")
            fb_sb = persist.tile([P, D_OUT], f32, tag="fb")
            ident = persist.tile([P, P], bf16, tag="ident")

            nc.sync.dma_start(fb_sb[:], fb_in[:])
            nc.sync.dma_start(c0_sb[:], c0_in[:])
            nc.sync.dma_start(c1_sb[:], c1_in[:])
            make_identity(nc, ident[:])

            def w_block(ob):
                wt = wlp.tile([P, D_IN], bf16, tag="w_bf16")
                mt = wlp.tile([P, D_IN], mybir.dt.uint8, tag="m_u8")
                nc.gpsimd.dma_start(wt[:], w_in[ob * P:(ob + 1) * P, :])   # f32->bf16
                nc.scalar.dma_start(mt[:], m_in[ob * P:(ob + 1) * P, :])
                w2 = wpp.tile([P, D_IN], bf16, tag="w2")
                nc.vector._custom_dve(
                    wprep, out=w2[:], in0=wt[:], in1=mt[:],
                    s0=c0_sb[:, ob:ob + 1], s1=c1_sb[:, ob:ob + 1], imm2=0.5)
                nc.sync.dma_start_transpose(wT[:, ob], w2[:])

            def x_block(j):
                xt = xlp.tile([P, D_IN], f32, tag="x_f32")
                nc.sync.dma_start(xt[:], x_in[j * P:(j + 1) * P, :])
                xb = xbp.tile([P, D_IN], bf16, tag="xb")
                nc.vector._custom_dve(
                    xprep, out=xb[:], in0=xt[:],
                    s0=float(inv_se), s1=MAGIC, imm2=QMAX)
                # transpose on the (otherwise idle-at-this-point) PE + ScalarE
                for b in range(NB_I):
                    tp = tpsum.tile([P, P], bf16, tag="xtp")
                    nc.tensor.transpose(tp[:], xb[:, b * P:(b + 1) * P], ident[:])
                    nc.scalar.copy(xqT[:, j, b, :], tp[:])

            # weight blocks for the first o-tile, then x, then the rest
            for ob in range(OB_PER_T):
                w_block(ob)
            for j in range(NJ):
                x_block(j)
            for ob in range(OB_PER_T, NB_O):
                w_block(ob)

            for t in range(NT):
                for j in range(NJ):
                    ps = psum.tile([P, OT], f32, tag="ps")
                    for b in range(NB_I):
                        nc.tensor.matmul(
                            ps[:],
                            xqT[:, j, b, :],
                            wT[:, t * OB_PER_T:(t + 1) * OB_PER_T, b, :],
                            start=(b == 0), stop=(b == NB_I - 1))
                    osb = ostage.tile([P, OT], f32, tag="osb")
                    nc.vector.tensor_add(
                        osb[:], ps[:], fb_sb[:, t * OT:(t + 1) * OT])
                    nc.scalar.dma_start(
                        out_o[j * P:(j + 1) * P, t * OT:(t + 1) * OT], osb[:])

    nc.compile()
    _CACHE[key] = nc
    return nc


def _in_maps(x, weight, mask_u8, c0, c1, fb):
    maps = []
    for c in range(N_CORES):
        maps.append({
            "x": np.ascontiguousarray(x[c * N_SHARD:(c + 1) * N_SHARD]),
            "w": weight,
            "m": mask_u8,
            "c0": c0,
            "c1": c1,
            "fb": fb,
        })
    return maps


def _host_consts(post_bin_scale, final_scale, final_bias, running_max):
    s = np.float32(running_max) / np.float32(QMAX)
    inv_se = np.float32(1.0) / (s + np.float32(EPS))
    sigma = np.float64(0.5) * (np.float64(s) + np.float64(EPS))
    c0_all = (sigma * final_scale.astype(np.float64)
              * post_bin_scale.reshape(-1).astype(np.float64)).astype(np.float32)
    c1_all = (np.float64(0.5) * sigma
              * final_scale.astype(np.float64)).astype(np.float32)
    # [o] -> [p, ob] with o = ob*128 + p
    c0 = np.ascontiguousarray(c0_all.reshape(NB_O, P).T)
    c1 = np.ascontiguousarray(c1_all.reshape(NB_O, P).T)
    fb = np.ascontiguousarray(
        np.broadcast_to(final_bias.astype(np.float32), (P, D_OUT)))
    return inv_se, c0, c1, fb


def kernel(x, weight, post_bin_scale, final_scale, final_bias, running_max,
           sprinkle_mask):
    from concourse.bass_utils import run_bass_kernel_spmd

    x = np.asarray(x, dtype=np.float32)
    weight = np.ascontiguousarray(np.asarray(weight, dtype=np.float32))
    mask_u8 = np.ascontiguousarray(np.asarray(sprinkle_mask)).view(np.uint8)
    inv_se, c0, c1, fb = _host_consts(
        np.asarray(post_bin_scale, dtype=np.float32),
        np.asarray(final_scale, dtype=np.float32),
        np.asarray(final_bias, dtype=np.float32),
        float(np.asarray(running_max)))

    nc = _build(inv_se)
    maps = _in_maps(x, weight, mask_u8, c0, c1, fb)
    res = run_bass_kernel_spmd(nc, maps, core_ids=list(range(N_CORES)))
    out = np.concatenate([res.results[c]["out"] for c in range(N_CORES)], axis=0)
    return out
